# revision 1
# baseline (speedup 1.0000x reference)
"""Trainium2 Bass kernel for nn_ComplexDilatedConv (complex pointwise -> PReLU -> LN ->
2x dilated depthwise conv (fused to one 5-tap complex conv) -> PReLU -> LN ->
complex pointwise + residual).

Sharding: pure data-parallel over batch (B=8 -> 8 cores). Each core processes one
batch element laid out as (128 partitions = [re(64); im(64)] channels, T=16384 free).
"""
import sys
sys.path.insert(0, '/opt/trn_rl_repo')
import numpy as np
import concourse.bacc as bacc
import concourse.mybir as mybir
from concourse import tile
from concourse.bass_utils import run_bass_kernel_spmd

dt = mybir.dt

B, CIN, T = 8, 64, 16384
MID = 128
ALPHA = 0.01
EPS = 1e-5
F = 512                  # chunk width (one PSUM bank of f32)
NCH = T // F             # 32 chunks
MARG = 16                # zero margin each side of v tiles (halo 4 needed)
OFFS = [-4, -2, 0, 2, 4]

_BF16 = None
_PROGRAM = None


def _np_bf16():
    global _BF16
    if _BF16 is None:
        _BF16 = dt.np(dt.bfloat16)
    return _BF16


def _build_program():
    A = mybir.AluOpType
    AFT = mybir.ActivationFunctionType
    nc = bacc.Bacc(None, target_bir_lowering=False)

    # ---------------- DRAM parameters ----------------
    x_in = nc.declare_dram_parameter("x_in", [128, T], dt.float32r, isOutput=False)
    w_in_r = nc.declare_dram_parameter("w_in_r", [128, 128], dt.float32r, isOutput=False)
    w_in_i = nc.declare_dram_parameter("w_in_i", [128, 128], dt.float32r, isOutput=False)
    b_in = nc.declare_dram_parameter("b_in", [128, 2], dt.float32, isOutput=False)
    a_re = nc.declare_dram_parameter("a_re", [128, 5], dt.float32, isOutput=False)
    a_im = nc.declare_dram_parameter("a_im", [128, 5], dt.float32, isOutput=False)
    na_im = nc.declare_dram_parameter("na_im", [128, 5], dt.float32, isOutput=False)
    edge_c = nc.declare_dram_parameter("edge_c", [128, 12], dt.float32, isOutput=False)
    db_c = nc.declare_dram_parameter("db_c", [128, 2], dt.float32, isOutput=False)
    w_out_a = nc.declare_dram_parameter("w_out_a", [128, 128], dt.bfloat16, isOutput=False)
    w_out_b = nc.declare_dram_parameter("w_out_b", [128, 128], dt.bfloat16, isOutput=False)
    b_fin = nc.declare_dram_parameter("b_fin", [128, 1], dt.float32, isOutput=False)
    out = nc.declare_dram_parameter("out", [128, T], dt.float32, isOutput=True)

    # ---------------- persistent SBUF ----------------
    u_r_h = nc.alloc_sbuf_tensor("u_r", [128, T], dt.bfloat16)          # 32K/p
    u_i_h = nc.alloc_sbuf_tensor("u_i", [128, T], dt.bfloat16)          # 32K/p
    v_r_h = nc.alloc_sbuf_tensor("v_r", [128, T + 2 * MARG], dt.bfloat16)
    v_i_h = nc.alloc_sbuf_tensor("v_i", [128, T + 2 * MARG], dt.bfloat16)
    win_r_h = nc.alloc_sbuf_tensor("win_r", [128, 128], dt.float32r)
    win_i_h = nc.alloc_sbuf_tensor("win_i", [128, 128], dt.float32r)
    wout_a_h = nc.alloc_sbuf_tensor("wout_a", [128, 128], dt.bfloat16)
    wout_b_h = nc.alloc_sbuf_tensor("wout_b", [128, 128], dt.bfloat16)
    b_in_h = nc.alloc_sbuf_tensor("b_in_sb", [128, 2], dt.float32)
    a_re_h = nc.alloc_sbuf_tensor("a_re_sb", [128, 5], dt.float32)
    a_im_h = nc.alloc_sbuf_tensor("a_im_sb", [128, 5], dt.float32)
    na_im_h = nc.alloc_sbuf_tensor("na_im_sb", [128, 5], dt.float32)
    edge_h = nc.alloc_sbuf_tensor("edge_sb", [128, 12], dt.float32)
    db_h = nc.alloc_sbuf_tensor("db_sb", [128, 2], dt.float32)
    bfin_h = nc.alloc_sbuf_tensor("bfin_sb", [128, 1], dt.float32)
    ones32_h = nc.alloc_sbuf_tensor("ones32", [128, 32], dt.bfloat16)
    onesrow4_h = nc.alloc_sbuf_tensor("onesrow4", [128, 128], dt.bfloat16)
    st_h = {}
    for nm in ("s1r", "s2r", "s1i", "s2i"):
        st_h[nm] = nc.alloc_sbuf_tensor("st_" + nm, [32, F], dt.float32)  # 2K/p ea
    abfin_h = {}
    ab4_h = {}
    for nm in ("Ar", "Br", "Ai", "Bi"):
        abfin_h[nm] = nc.alloc_sbuf_tensor("abf_" + nm, [32, F], dt.bfloat16)  # 1K/p
        ab4_h[nm] = nc.alloc_sbuf_tensor("ab4_" + nm, [128, 8 * F], dt.bfloat16)  # 8K/p

    # ---------------- preamble ----------------
    nc.gpsimd.memset(ones32_h.ap(), 1.0)
    nc.gpsimd.memset(onesrow4_h.ap(), 1.0)
    nc.gpsimd.memset(v_r_h.ap()[:, 0:MARG], 0.0)
    nc.gpsimd.memset(v_r_h.ap()[:, T + MARG:], 0.0)
    nc.gpsimd.memset(v_i_h.ap()[:, 0:MARG], 0.0)
    nc.gpsimd.memset(v_i_h.ap()[:, T + MARG:], 0.0)
    with nc.semaphore("pre_sem") as pre:
        n = 0
        for dst, src in [
            (win_r_h.ap(), w_in_r[:]), (win_i_h.ap(), w_in_i[:]),
            (wout_a_h.ap(), w_out_a[:]), (wout_b_h.ap(), w_out_b[:]),
            (b_in_h.ap(), b_in[:]), (a_re_h.ap(), a_re[:]),
            (a_im_h.ap(), a_im[:]), (na_im_h.ap(), na_im[:]),
            (edge_h.ap(), edge_c[:]), (db_h.ap(), db_c[:]), (bfin_h.ap(), b_fin[:]),
        ]:
            nc.sync.dma_start(dst, src).then_inc(pre, 16)
            n += 1
        nc.sync.wait_ge(pre, 16 * n)
    nc.all_engine_barrier()

    u = {"r": u_r_h.ap(), "i": u_i_h.ap()}
    v = {"r": v_r_h.ap(), "i": v_i_h.ap()}
    win = {"r": win_r_h.ap(), "i": win_i_h.ap()}
    ones32 = ones32_h.ap()
    onesrow4 = onesrow4_h.ap()

    def ln_stats(sq_pool, ps_pool, src):
        """Per-chunk: squares + 4 ones-MMs + drains + gather DMAs into st tiles."""
        for k in range(NCH):
            cs = slice(k * F, (k + 1) * F)
            for part, dr_act in (("r", True), ("i", False)):
                uc = src[part][:, cs]
                sqt = sq_pool.tile([128, F], dt.bfloat16, tag="sq" + part)
                nc.vector.tensor_tensor(sqt[:], uc, uc, A.mult)
                sp1 = ps_pool.tile([32, F], dt.float32, tag="sp1" + part)
                sp2 = ps_pool.tile([32, F], dt.float32, tag="sp2" + part)
                nc.tensor.matmul(sp1[:, :], ones32[:], uc, start=True, stop=True)
                nc.tensor.matmul(sp2[:, :], ones32[:], sqt[:], start=True, stop=True)
                d1 = sq_pool.tile([32, F], dt.float32, tag="d1" + part)
                d2 = sq_pool.tile([32, F], dt.float32, tag="d2" + part)
                if dr_act:
                    nc.scalar.copy(d1[:, :], sp1[:, :])
                    nc.scalar.copy(d2[:, :], sp2[:, :])
                else:
                    nc.vector.tensor_copy(d1[:, :], sp1[:, :])
                    nc.vector.tensor_copy(d2[:, :], sp2[:, :])
                nc.sync.dma_start(st_h["s1" + part].ap()[k:k + 1, :], d1[0:1, :])
                nc.sync.dma_start(st_h["s2" + part].ap()[k:k + 1, :], d2[0:1, :])

    def ln_finalize(pool):
        """Batched: A = rsqrt(var+eps), B = -mu*A; scatter into 4-quadrant ab4 layout."""
        for part in ("r", "i"):
            s1 = st_h["s1" + part].ap()
            s2 = st_h["s2" + part].ap()
            mu = pool.tile([32, F], dt.float32, tag="mu")
            m2e = pool.tile([32, F], dt.float32, tag="m2e")
            var = pool.tile([32, F], dt.float32, tag="var")
            rec = pool.tile([32, F], dt.float32, tag="rec")
            at = abfin_h["A" + part].ap()
            bt = abfin_h["B" + part].ap()
            nc.vector.tensor_scalar(mu[:], s1[:], 1.0 / MID, None, A.mult)
            nc.vector.scalar_tensor_tensor(m2e[:], mu[:], -1.0, mu[:], A.mult, A.mult)
            nc.vector.tensor_scalar(var[:], s2[:], 1.0 / MID, EPS, A.mult, A.add)
            nc.vector.tensor_tensor(var[:], var[:], m2e[:], A.add)
            nc.vector.reciprocal(rec[:], var[:])
            nc.scalar.activation(at[:], rec[:], AFT.Sqrt)
            nc.vector.scalar_tensor_tensor(bt[:], mu[:], -1.0, at[:], A.mult, A.mult)
            # chunk k -> (partition 32*(k%4), free slot k//4)
            for nm, src_ap in (("A" + part, at), ("B" + part, bt)):
                for a in range(4):
                    dst = ab4_h[nm].ap()[32 * a:32 * a + 1, :].rearrange(
                        "p (s f) -> p s f", s=8)
                    nc.sync.dma_start(dst, src_ap[a:a + 29:4, :])

    def bcast(ps_pool, nm, k):
        """(128,F) psum broadcast of chunk-k row of ab4[nm]."""
        b = 32 * (k % 4)
        s = k // 4
        bc = ps_pool.tile([128, F], dt.float32, tag="bc" + nm)
        nc.tensor.matmul(bc[:], onesrow4[b:b + 1, :],
                         ab4_h[nm].ap()[b:b + 1, s * F:(s + 1) * F],
                         start=True, stop=True, tile_position=(b, 0))
        return bc

    def apply_ln(ps_pool, src_ap, dst_ap, part, k):
        """dst = src * A_bc + B_bc for chunk k."""
        bca = bcast(ps_pool, "A" + part, k)
        bcb = bcast(ps_pool, "B" + part, k)
        nc.vector.scalar_tensor_tensor(dst_ap, src_ap, 1.0, bca[:], A.bypass, A.mult)
        nc.vector.scalar_tensor_tensor(dst_ap, dst_ap, 1.0, bcb[:], A.bypass, A.add)

    with tile.TileContext(nc) as tc:
        # ---------- P1: in-pointwise + PReLU1 + LN1 stats ----------
        with tc.tile_pool(name="p1sq", bufs=2) as sqp, \
             tc.tile_pool(name="p1x", bufs=3) as xp, \
             tc.tile_pool(name="p1st", bufs=1, space="PSUM") as stp, \
             tc.tile_pool(name="p1ps", bufs=2, space="PSUM") as psp:
            for k in range(NCH):
                cs = slice(k * F, (k + 1) * F)
                xs = xp.tile([128, F], dt.float32r, tag="xs")
                nc.sync.dma_start(xs[:], x_in[:, cs])
                for part, bcol in (("r", 0), ("i", 1)):
                    yp = psp.tile([128, F], dt.float32, tag="y" + part)
                    nc.tensor.matmul(yp[:], win[part][:], xs[:], start=True, stop=True)
                    nc.scalar.activation(u[part][:, cs], yp[:], AFT.Lrelu,
                                         bias=b_in_h.ap()[:, bcol:bcol + 1],
                                         scale=1.0, alpha=ALPHA)
            ln_stats(sqp, stp, u)
        with tc.tile_pool(name="f1", bufs=1) as fp:
            ln_finalize(fp)

        # ---------- P2: LN1 apply -> v ; dconv ; PReLU2 -> u2 ; LN2 stats ----------
        with tc.tile_pool(name="p2sq", bufs=2) as sqp, \
             tc.tile_pool(name="p2ps", bufs=1, space="PSUM") as psp, \
             tc.tile_pool(name="p2z", bufs=2) as zp:
            for k in range(NCH):
                for part in ("r", "i"):
                    dst = v[part][:, MARG + k * F: MARG + (k + 1) * F]
                    apply_ln(psp, u[part][:, k * F:(k + 1) * F], dst, part, k)
            for k in range(NCH):
                base = MARG + k * F
                zr = zp.tile([128, F], dt.bfloat16, tag="zr")
                zi = zp.tile([128, F], dt.bfloat16, tag="zi")
                for j in range(5):
                    off = OFFS[j]
                    vr_s = v["r"][:, base + off: base + off + F]
                    vi_s = v["i"][:, base + off: base + off + F]
                    ar_c = a_re_h.ap()[:, j:j + 1]
                    ai_c = a_im_h.ap()[:, j:j + 1]
                    nai_c = na_im_h.ap()[:, j:j + 1]
                    if j == 0:
                        nc.vector.tensor_scalar(zr[:], vr_s, ar_c, None, A.mult)
                        nc.vector.tensor_scalar(zi[:], vr_s, ai_c, None, A.mult)
                    else:
                        nc.vector.scalar_tensor_tensor(zr[:], vr_s, ar_c, zr[:],
                                                       A.mult, A.add)
                        nc.vector.scalar_tensor_tensor(zi[:], vr_s, ai_c, zi[:],
                                                       A.mult, A.add)
                    nc.vector.scalar_tensor_tensor(zr[:], vi_s, nai_c, zr[:],
                                                   A.mult, A.add)
                    nc.vector.scalar_tensor_tensor(zi[:], vi_s, ar_c, zi[:],
                                                   A.mult, A.add)
                if k == 0 or k == NCH - 1:
                    if k == 0:
                        col = slice(0, 2)
                        vr_e = v["r"][:, MARG + 0: MARG + 2]
                        vi_e = v["i"][:, MARG + 0: MARG + 2]
                        be = 0
                    else:
                        col = slice(F - 2, F)
                        vr_e = v["r"][:, MARG + T - 2: MARG + T]
                        vi_e = v["i"][:, MARG + T - 2: MARG + T]
                        be = 6
                    e = edge_h.ap()
                    # zr += (-ew_r)*vr + (+ew_i)*vi + (-eb_r)
                    nc.vector.scalar_tensor_tensor(zr[:, col], vr_e, e[:, be:be + 1],
                                                   zr[:, col], A.mult, A.add)
                    nc.vector.scalar_tensor_tensor(zr[:, col], vi_e, e[:, be + 1:be + 2],
                                                   zr[:, col], A.mult, A.add)
                    nc.vector.tensor_scalar(zr[:, col], zr[:, col],
                                            e[:, be + 4:be + 5], None, A.add)
                    # zi += (-ew_i)*vr + (-ew_r)*vi + (-eb_i)
                    nc.vector.scalar_tensor_tensor(zi[:, col], vr_e, e[:, be + 2:be + 3],
                                                   zi[:, col], A.mult, A.add)
                    nc.vector.scalar_tensor_tensor(zi[:, col], vi_e, e[:, be:be + 1],
                                                   zi[:, col], A.mult, A.add)
                    nc.vector.tensor_scalar(zi[:, col], zi[:, col],
                                            e[:, be + 5:be + 6], None, A.add)
                cs = slice(k * F, (k + 1) * F)
                nc.scalar.activation(u["r"][:, cs], zr[:], AFT.Lrelu,
                                     bias=db_h.ap()[:, 0:1], scale=1.0, alpha=ALPHA)
                nc.scalar.activation(u["i"][:, cs], zi[:], AFT.Lrelu,
                                     bias=db_h.ap()[:, 1:2], scale=1.0, alpha=ALPHA)
            ln_stats(sqp, psp, u)
        with tc.tile_pool(name="f2", bufs=1) as fp:
            ln_finalize(fp)

        # ---------- P3: LN2 apply -> v2 ; out-pointwise ; residual+bias ; store ----------
        with tc.tile_pool(name="p3v", bufs=2) as vp, \
             tc.tile_pool(name="p3ps", bufs=1, space="PSUM") as psp, \
             tc.tile_pool(name="p3po", bufs=2, space="PSUM") as pop, \
             tc.tile_pool(name="p3o", bufs=3) as op:
            for k in range(NCH):
                cs = slice(k * F, (k + 1) * F)
                v2r = vp.tile([128, F], dt.bfloat16, tag="v2r")
                v2i = vp.tile([128, F], dt.bfloat16, tag="v2i")
                apply_ln(psp, u["r"][:, cs], v2r[:], "r", k)
                apply_ln(psp, u["i"][:, cs], v2i[:], "i", k)
                po = pop.tile([128, F], dt.float32, tag="po")
                nc.tensor.matmul(po[:], wout_a_h.ap()[:], v2r[:], start=True, stop=False)
                nc.tensor.matmul(po[:], wout_b_h.ap()[:], v2i[:], start=False, stop=True)
                xres = op.tile([128, F], dt.float32r, tag="xres")
                nc.sync.dma_start(xres[:], x_in[:, cs])
                osb = op.tile([128, F], dt.float32, tag="osb")
                nc.vector.scalar_tensor_tensor(osb[:], po[:], bfin_h.ap()[:, 0:1],
                                               xres[:].bitcast(dt.float32),
                                               A.add, A.add)
                nc.sync.dma_start(out[:, cs], osb[:])

    nc.finalize()
    return nc


def _get_program():
    global _PROGRAM
    if _PROGRAM is None:
        _PROGRAM = _build_program()
    return _PROGRAM


def _prep_weights(inp):
    bf16 = _np_bf16()
    w_in_re = np.asarray(inp["w_in_re"], np.float64)
    w_in_im = np.asarray(inp["w_in_im"], np.float64)
    ln_in_w = np.asarray(inp["ln_in_w"], np.float64)
    ln_in_b = np.asarray(inp["ln_in_b"], np.float64)
    dw = np.asarray(inp["dw_re"], np.float64) + 1j * np.asarray(inp["dw_im"], np.float64)
    db = np.asarray(inp["db_re"], np.float64) + 1j * np.asarray(inp["db_im"], np.float64)
    ln_out_w = np.asarray(inp["ln_out_w"], np.float64)
    ln_out_b = np.asarray(inp["ln_out_b"], np.float64)
    w_out = np.asarray(inp["w_out_re"], np.float64) + 1j * np.asarray(inp["w_out_im"], np.float64)
    b_out = np.asarray(inp["b_out_re"], np.float64) + 1j * np.asarray(inp["b_out_im"], np.float64)

    lin_r = np.zeros((128, 128), np.float64)
    lin_r[0:64, :] = w_in_re.T
    lin_r[64:128, :] = -w_in_im.T
    lin_i = np.zeros((128, 128), np.float64)
    lin_i[0:64, :] = w_in_im.T
    lin_i[64:128, :] = w_in_re.T
    b_in = np.stack([np.asarray(inp["b_in_re"], np.float64),
                     np.asarray(inp["b_in_im"], np.float64)], axis=1)

    w1, w2 = dw[0, :, 0, :], dw[1, :, 0, :]
    a_taps = np.stack([np.convolve(w2[c], w1[c]) for c in range(MID)])
    a_eff = a_taps * ln_in_w[:, None]
    bias_d = a_taps.sum(1) * ln_in_b + w2.sum(1) * db[0] + db[1]
    e_lo = w2[:, 0] * w1[:, 2]
    e_hi = w2[:, 2] * w1[:, 0]
    e_lo_w = e_lo * ln_in_w
    e_hi_w = e_hi * ln_in_w
    e_lo_b = e_lo * ln_in_b + w2[:, 0] * db[0]
    e_hi_b = e_hi * ln_in_b + w2[:, 2] * db[0]
    edge = np.zeros((128, 12), np.float64)
    edge[:, 0] = -e_lo_w.real
    edge[:, 1] = e_lo_w.imag
    edge[:, 2] = -e_lo_w.imag
    edge[:, 4] = -e_lo_b.real
    edge[:, 5] = -e_lo_b.imag
    edge[:, 6] = -e_hi_w.real
    edge[:, 7] = e_hi_w.imag
    edge[:, 8] = -e_hi_w.imag
    edge[:, 10] = -e_hi_b.real
    edge[:, 11] = -e_hi_b.imag
    db_cc = np.stack([bias_d.real, bias_d.imag], axis=1)

    w_out2 = w_out * ln_out_w[None, :]
    bias_out = w_out @ ln_out_b + b_out
    lA = np.zeros((128, 128), np.float64)
    lA[:, 0:64] = w_out2.real.T
    lA[:, 64:128] = w_out2.imag.T
    lB = np.zeros((128, 128), np.float64)
    lB[:, 0:64] = -w_out2.imag.T
    lB[:, 64:128] = w_out2.real.T
    b_fin = np.concatenate([bias_out.real, bias_out.imag])[:, None]

    return {
        "w_in_r": lin_r.astype(np.float32), "w_in_i": lin_i.astype(np.float32),
        "b_in": b_in.astype(np.float32),
        "a_re": a_eff.real.astype(np.float32), "a_im": a_eff.imag.astype(np.float32),
        "na_im": (-a_eff.imag).astype(np.float32),
        "edge_c": edge.astype(np.float32), "db_c": db_cc.astype(np.float32),
        "w_out_a": lA.astype(bf16), "w_out_b": lB.astype(bf16),
        "b_fin": b_fin.astype(np.float32),
    }


def kernel(**inputs):
    nc = _get_program()
    wmap = _prep_weights(inputs)
    x_re = np.asarray(inputs["x_re"], np.float32)
    x_im = np.asarray(inputs["x_im"], np.float32)

    in_maps = []
    for b in range(B):
        x_st = np.concatenate([x_re[b], x_im[b]], axis=0).astype(np.float32)
        m = dict(wmap)
        m["x_in"] = x_st
        in_maps.append(m)

    res = run_bass_kernel_spmd(nc, in_maps, core_ids=list(range(B)))
    out = np.empty((2, B, CIN, T), np.float32)
    for b in range(B):
        o = res.results[b]["out"]
        out[0, b] = o[0:64]
        out[1, b] = o[64:128]
    return out



# revision 2
# speedup vs baseline: 56.5852x; 56.5852x over previous
"""Trainium2 Bass kernel for nn_ComplexDilatedConv (complex pointwise -> PReLU -> LN ->
2x dilated depthwise conv (fused to one 5-tap complex conv) -> PReLU -> LN ->
complex pointwise + residual).

Sharding: pure data-parallel over batch (B=8 -> 8 cores). Each core processes one
batch element laid out as (128 partitions = [re(64); im(64)] channels, T=16384 free).
"""
import sys
sys.path.insert(0, '/opt/trn_rl_repo')
import numpy as np
import concourse.bacc as bacc
import concourse.mybir as mybir
from concourse import tile
from concourse.bass_utils import run_bass_kernel_spmd

dt = mybir.dt

B, CIN, T = 8, 64, 16384
MID = 128
ALPHA = 0.01
EPS = 1e-5
F = 512                  # chunk width (one PSUM bank of f32)
NCH = T // F             # 32 chunks
MARG = 16                # zero margin each side of v tiles (halo 4 needed)
OFFS = [-4, -2, 0, 2, 4]

_BF16 = None
_PROGRAM = None


def _np_bf16():
    global _BF16
    if _BF16 is None:
        _BF16 = dt.np(dt.bfloat16)
    return _BF16


def _build_program():
    A = mybir.AluOpType
    AFT = mybir.ActivationFunctionType
    nc = bacc.Bacc(None, target_bir_lowering=False)

    # ---------------- DRAM parameters ----------------
    x_in = nc.declare_dram_parameter("x_in", [128, T], dt.float32r, isOutput=False)
    w_in_r = nc.declare_dram_parameter("w_in_r", [128, 128], dt.float32r, isOutput=False)
    w_in_i = nc.declare_dram_parameter("w_in_i", [128, 128], dt.float32r, isOutput=False)
    b_in = nc.declare_dram_parameter("b_in", [128, 2], dt.float32, isOutput=False)
    a_re = nc.declare_dram_parameter("a_re", [128, 5], dt.float32, isOutput=False)
    a_im = nc.declare_dram_parameter("a_im", [128, 5], dt.float32, isOutput=False)
    na_im = nc.declare_dram_parameter("na_im", [128, 5], dt.float32, isOutput=False)
    edge_c = nc.declare_dram_parameter("edge_c", [128, 12], dt.float32, isOutput=False)
    db_c = nc.declare_dram_parameter("db_c", [128, 2], dt.float32, isOutput=False)
    w_out_a = nc.declare_dram_parameter("w_out_a", [128, 128], dt.bfloat16, isOutput=False)
    w_out_b = nc.declare_dram_parameter("w_out_b", [128, 128], dt.bfloat16, isOutput=False)
    b_fin = nc.declare_dram_parameter("b_fin", [128, 1], dt.float32, isOutput=False)
    out = nc.declare_dram_parameter("out", [128, T], dt.float32, isOutput=True)

    # ---------------- persistent SBUF ----------------
    u_r_h = nc.alloc_sbuf_tensor("u_r", [128, T], dt.bfloat16)          # 32K/p
    u_i_h = nc.alloc_sbuf_tensor("u_i", [128, T], dt.bfloat16)          # 32K/p
    v_r_h = nc.alloc_sbuf_tensor("v_r", [128, T + 2 * MARG], dt.bfloat16)
    v_i_h = nc.alloc_sbuf_tensor("v_i", [128, T + 2 * MARG], dt.bfloat16)
    win_r_h = nc.alloc_sbuf_tensor("win_r", [128, 128], dt.float32r)
    win_i_h = nc.alloc_sbuf_tensor("win_i", [128, 128], dt.float32r)
    wout_a_h = nc.alloc_sbuf_tensor("wout_a", [128, 128], dt.bfloat16)
    wout_b_h = nc.alloc_sbuf_tensor("wout_b", [128, 128], dt.bfloat16)
    b_in_h = nc.alloc_sbuf_tensor("b_in_sb", [128, 2], dt.float32)
    a_re_h = nc.alloc_sbuf_tensor("a_re_sb", [128, 5], dt.float32)
    a_im_h = nc.alloc_sbuf_tensor("a_im_sb", [128, 5], dt.float32)
    na_im_h = nc.alloc_sbuf_tensor("na_im_sb", [128, 5], dt.float32)
    edge_h = nc.alloc_sbuf_tensor("edge_sb", [128, 12], dt.float32)
    db_h = nc.alloc_sbuf_tensor("db_sb", [128, 2], dt.float32)
    bfin_h = nc.alloc_sbuf_tensor("bfin_sb", [128, 1], dt.float32)
    ones32_h = nc.alloc_sbuf_tensor("ones32", [128, 32], dt.bfloat16)
    onesrow4_h = nc.alloc_sbuf_tensor("onesrow4", [128, 128], dt.bfloat16)
    st_h = {}
    for nm in ("s1r", "s2r", "s1i", "s2i"):
        st_h[nm] = nc.alloc_sbuf_tensor("st_" + nm, [32, F], dt.float32)  # 2K/p ea
    abfin_h = {}
    ab4_h = {}
    for nm in ("Ar", "Br", "Ai", "Bi"):
        abfin_h[nm] = nc.alloc_sbuf_tensor("abf_" + nm, [32, F], dt.bfloat16)  # 1K/p
        ab4_h[nm] = nc.alloc_sbuf_tensor("ab4_" + nm, [128, 8 * F], dt.bfloat16)  # 8K/p

    # ---------------- preamble ----------------
    nc.gpsimd.memset(ones32_h.ap(), 1.0)
    nc.gpsimd.memset(onesrow4_h.ap(), 1.0)
    nc.gpsimd.memset(v_r_h.ap()[:, 0:MARG], 0.0)
    nc.gpsimd.memset(v_r_h.ap()[:, T + MARG:], 0.0)
    nc.gpsimd.memset(v_i_h.ap()[:, 0:MARG], 0.0)
    nc.gpsimd.memset(v_i_h.ap()[:, T + MARG:], 0.0)
    with nc.semaphore("pre_sem") as pre:
        n = 0
        for dst, src in [
            (win_r_h.ap(), w_in_r[:]), (win_i_h.ap(), w_in_i[:]),
            (wout_a_h.ap(), w_out_a[:]), (wout_b_h.ap(), w_out_b[:]),
            (b_in_h.ap(), b_in[:]), (a_re_h.ap(), a_re[:]),
            (a_im_h.ap(), a_im[:]), (na_im_h.ap(), na_im[:]),
            (edge_h.ap(), edge_c[:]), (db_h.ap(), db_c[:]), (bfin_h.ap(), b_fin[:]),
        ]:
            nc.sync.dma_start(dst, src).then_inc(pre, 16)
            n += 1
        nc.sync.wait_ge(pre, 16 * n)
    nc.all_engine_barrier()

    u = {"r": u_r_h.ap(), "i": u_i_h.ap()}
    v = {"r": v_r_h.ap(), "i": v_i_h.ap()}
    win = {"r": win_r_h.ap(), "i": win_i_h.ap()}
    ones32 = ones32_h.ap()
    onesrow4 = onesrow4_h.ap()

    def ln_stats(sq_pool, ps_pool, src):
        """Per-chunk: squares + 4 ones-MMs + drains + gather DMAs into st tiles."""
        for k in range(NCH):
            cs = slice(k * F, (k + 1) * F)
            for part, dr_act in (("r", True), ("i", False)):
                uc = src[part][:, cs]
                sqt = sq_pool.tile([128, F], dt.bfloat16, tag="sq" + part)
                nc.vector.tensor_tensor(sqt[:], uc, uc, A.mult)
                sp1 = ps_pool.tile([32, F], dt.float32, tag="sp1" + part)
                sp2 = ps_pool.tile([32, F], dt.float32, tag="sp2" + part)
                nc.tensor.matmul(sp1[:, :], ones32[:], uc, start=True, stop=True)
                nc.tensor.matmul(sp2[:, :], ones32[:], sqt[:], start=True, stop=True)
                d1 = sq_pool.tile([32, F], dt.float32, tag="d1" + part)
                d2 = sq_pool.tile([32, F], dt.float32, tag="d2" + part)
                if dr_act:
                    nc.scalar.copy(d1[:, :], sp1[:, :])
                    nc.scalar.copy(d2[:, :], sp2[:, :])
                else:
                    nc.vector.tensor_copy(d1[:, :], sp1[:, :])
                    nc.vector.tensor_copy(d2[:, :], sp2[:, :])
                nc.sync.dma_start(st_h["s1" + part].ap()[k:k + 1, :], d1[0:1, :])
                nc.sync.dma_start(st_h["s2" + part].ap()[k:k + 1, :], d2[0:1, :])

    def ln_finalize(pool):
        """Batched: A = rsqrt(var+eps), B = -mu*A; scatter into 4-quadrant ab4 layout."""
        for part in ("r", "i"):
            s1 = st_h["s1" + part].ap()
            s2 = st_h["s2" + part].ap()
            mu = pool.tile([32, F], dt.float32, tag="mu")
            m2e = pool.tile([32, F], dt.float32, tag="m2e")
            var = pool.tile([32, F], dt.float32, tag="var")
            rec = pool.tile([32, F], dt.float32, tag="rec")
            at = abfin_h["A" + part].ap()
            bt = abfin_h["B" + part].ap()
            nc.vector.tensor_scalar(mu[:], s1[:], 1.0 / MID, None, A.mult)
            nc.vector.scalar_tensor_tensor(m2e[:], mu[:], -1.0, mu[:], A.mult, A.mult)
            nc.vector.tensor_scalar(var[:], s2[:], 1.0 / MID, EPS, A.mult, A.add)
            nc.vector.tensor_tensor(var[:], var[:], m2e[:], A.add)
            nc.vector.reciprocal(rec[:], var[:])
            nc.scalar.activation(at[:], rec[:], AFT.Sqrt)
            nc.vector.scalar_tensor_tensor(bt[:], mu[:], -1.0, at[:], A.mult, A.mult)
            # chunk k -> (partition 32*(k%4), free slot k//4)
            for nm, src_ap in (("A" + part, at), ("B" + part, bt)):
                for a in range(4):
                    dst = ab4_h[nm].ap()[32 * a:32 * a + 1, :].rearrange(
                        "p (s f) -> p s f", s=8)
                    nc.sync.dma_start(dst, src_ap[a:a + 29:4, :])

    def bcast(ps_pool, nm, k):
        """(128,F) psum broadcast of chunk-k row of ab4[nm]."""
        b = 32 * (k % 4)
        s = k // 4
        bc = ps_pool.tile([128, F], dt.float32, tag="bc" + nm)
        nc.tensor.matmul(bc[:], onesrow4[b:b + 1, :],
                         ab4_h[nm].ap()[b:b + 1, s * F:(s + 1) * F],
                         start=True, stop=True, tile_position=(b, 0))
        return bc

    def apply_ln(ps_pool, src_ap, dst_ap, part, k):
        """dst = src * A_bc + B_bc for chunk k."""
        bca = bcast(ps_pool, "A" + part, k)
        bcb = bcast(ps_pool, "B" + part, k)
        nc.vector.scalar_tensor_tensor(dst_ap, src_ap, 1.0, bca[:], A.bypass, A.mult)
        nc.vector.scalar_tensor_tensor(dst_ap, dst_ap, 1.0, bcb[:], A.bypass, A.add)

    with tile.TileContext(nc) as tc:
        # ---------- P1: in-pointwise + PReLU1 + LN1 stats ----------
        with tc.tile_pool(name="p1sq", bufs=2) as sqp, \
             tc.tile_pool(name="p1x", bufs=3) as xp, \
             tc.tile_pool(name="p1st", bufs=1, space="PSUM") as stp, \
             tc.tile_pool(name="p1ps", bufs=2, space="PSUM") as psp:
            for k in range(NCH):
                cs = slice(k * F, (k + 1) * F)
                xs = xp.tile([128, F], dt.float32r, tag="xs")
                nc.sync.dma_start(xs[:], x_in[:, cs])
                for part, bcol in (("r", 0), ("i", 1)):
                    yp = psp.tile([128, F], dt.float32, tag="y" + part)
                    nc.tensor.matmul(yp[:], win[part][:], xs[:], start=True, stop=True)
                    nc.scalar.activation(u[part][:, cs], yp[:], AFT.Lrelu,
                                         bias=b_in_h.ap()[:, bcol:bcol + 1],
                                         scale=1.0, alpha=ALPHA)
            ln_stats(sqp, stp, u)
        with tc.tile_pool(name="f1", bufs=1) as fp:
            ln_finalize(fp)

        # ---------- P2: LN1 apply -> v ; dconv ; PReLU2 -> u2 ; LN2 stats ----------
        with tc.tile_pool(name="p2sq", bufs=2) as sqp, \
             tc.tile_pool(name="p2ps", bufs=1, space="PSUM") as psp, \
             tc.tile_pool(name="p2z", bufs=2) as zp:
            for k in range(NCH):
                for part in ("r", "i"):
                    dst = v[part][:, MARG + k * F: MARG + (k + 1) * F]
                    apply_ln(psp, u[part][:, k * F:(k + 1) * F], dst, part, k)
            for k in range(NCH):
                base = MARG + k * F
                zr = zp.tile([128, F], dt.bfloat16, tag="zr")
                zi = zp.tile([128, F], dt.bfloat16, tag="zi")
                for j in range(5):
                    off = OFFS[j]
                    vr_s = v["r"][:, base + off: base + off + F]
                    vi_s = v["i"][:, base + off: base + off + F]
                    ar_c = a_re_h.ap()[:, j:j + 1]
                    ai_c = a_im_h.ap()[:, j:j + 1]
                    nai_c = na_im_h.ap()[:, j:j + 1]
                    if j == 0:
                        nc.vector.tensor_scalar(zr[:], vr_s, ar_c, None, A.mult)
                        nc.vector.tensor_scalar(zi[:], vr_s, ai_c, None, A.mult)
                    else:
                        nc.vector.scalar_tensor_tensor(zr[:], vr_s, ar_c, zr[:],
                                                       A.mult, A.add)
                        nc.vector.scalar_tensor_tensor(zi[:], vr_s, ai_c, zi[:],
                                                       A.mult, A.add)
                    nc.vector.scalar_tensor_tensor(zr[:], vi_s, nai_c, zr[:],
                                                   A.mult, A.add)
                    nc.vector.scalar_tensor_tensor(zi[:], vi_s, ar_c, zi[:],
                                                   A.mult, A.add)
                if k == 0 or k == NCH - 1:
                    if k == 0:
                        col = slice(0, 2)
                        vr_e = v["r"][:, MARG + 0: MARG + 2]
                        vi_e = v["i"][:, MARG + 0: MARG + 2]
                        be = 0
                    else:
                        col = slice(F - 2, F)
                        vr_e = v["r"][:, MARG + T - 2: MARG + T]
                        vi_e = v["i"][:, MARG + T - 2: MARG + T]
                        be = 6
                    e = edge_h.ap()
                    # zr += (-ew_r)*vr + (+ew_i)*vi + (-eb_r)
                    nc.vector.scalar_tensor_tensor(zr[:, col], vr_e, e[:, be:be + 1],
                                                   zr[:, col], A.mult, A.add)
                    nc.vector.scalar_tensor_tensor(zr[:, col], vi_e, e[:, be + 1:be + 2],
                                                   zr[:, col], A.mult, A.add)
                    nc.vector.tensor_scalar(zr[:, col], zr[:, col],
                                            e[:, be + 4:be + 5], None, A.add)
                    # zi += (-ew_i)*vr + (-ew_r)*vi + (-eb_i)
                    nc.vector.scalar_tensor_tensor(zi[:, col], vr_e, e[:, be + 2:be + 3],
                                                   zi[:, col], A.mult, A.add)
                    nc.vector.scalar_tensor_tensor(zi[:, col], vi_e, e[:, be:be + 1],
                                                   zi[:, col], A.mult, A.add)
                    nc.vector.tensor_scalar(zi[:, col], zi[:, col],
                                            e[:, be + 5:be + 6], None, A.add)
                cs = slice(k * F, (k + 1) * F)
                nc.scalar.activation(u["r"][:, cs], zr[:], AFT.Lrelu,
                                     bias=db_h.ap()[:, 0:1], scale=1.0, alpha=ALPHA)
                nc.scalar.activation(u["i"][:, cs], zi[:], AFT.Lrelu,
                                     bias=db_h.ap()[:, 1:2], scale=1.0, alpha=ALPHA)
            ln_stats(sqp, psp, u)
        with tc.tile_pool(name="f2", bufs=1) as fp:
            ln_finalize(fp)

        # ---------- P3: LN2 apply -> v2 ; out-pointwise ; residual+bias ; store ----------
        with tc.tile_pool(name="p3v", bufs=2) as vp, \
             tc.tile_pool(name="p3ps", bufs=1, space="PSUM") as psp, \
             tc.tile_pool(name="p3po", bufs=2, space="PSUM") as pop, \
             tc.tile_pool(name="p3o", bufs=3) as op:
            for k in range(NCH):
                cs = slice(k * F, (k + 1) * F)
                v2r = vp.tile([128, F], dt.bfloat16, tag="v2r")
                v2i = vp.tile([128, F], dt.bfloat16, tag="v2i")
                apply_ln(psp, u["r"][:, cs], v2r[:], "r", k)
                apply_ln(psp, u["i"][:, cs], v2i[:], "i", k)
                po = pop.tile([128, F], dt.float32, tag="po")
                nc.tensor.matmul(po[:], wout_a_h.ap()[:], v2r[:], start=True, stop=False)
                nc.tensor.matmul(po[:], wout_b_h.ap()[:], v2i[:], start=False, stop=True)
                xres = op.tile([128, F], dt.float32r, tag="xres")
                nc.sync.dma_start(xres[:], x_in[:, cs])
                osb = op.tile([128, F], dt.float32, tag="osb")
                nc.vector.scalar_tensor_tensor(osb[:], po[:], bfin_h.ap()[:, 0:1],
                                               xres[:].bitcast(dt.float32),
                                               A.add, A.add)
                nc.sync.dma_start(out[:, cs], osb[:])

    nc.finalize()
    return nc


def _get_program():
    global _PROGRAM
    if _PROGRAM is None:
        _PROGRAM = _build_program()
    return _PROGRAM


def _prep_weights(inp):
    bf16 = _np_bf16()
    w_in_re = np.asarray(inp["w_in_re"], np.float64)
    w_in_im = np.asarray(inp["w_in_im"], np.float64)
    ln_in_w = np.asarray(inp["ln_in_w"], np.float64)
    ln_in_b = np.asarray(inp["ln_in_b"], np.float64)
    dw = np.asarray(inp["dw_re"], np.float64) + 1j * np.asarray(inp["dw_im"], np.float64)
    db = np.asarray(inp["db_re"], np.float64) + 1j * np.asarray(inp["db_im"], np.float64)
    ln_out_w = np.asarray(inp["ln_out_w"], np.float64)
    ln_out_b = np.asarray(inp["ln_out_b"], np.float64)
    w_out = np.asarray(inp["w_out_re"], np.float64) + 1j * np.asarray(inp["w_out_im"], np.float64)
    b_out = np.asarray(inp["b_out_re"], np.float64) + 1j * np.asarray(inp["b_out_im"], np.float64)

    lin_r = np.zeros((128, 128), np.float64)
    lin_r[0:64, :] = w_in_re.T
    lin_r[64:128, :] = -w_in_im.T
    lin_i = np.zeros((128, 128), np.float64)
    lin_i[0:64, :] = w_in_im.T
    lin_i[64:128, :] = w_in_re.T
    b_in = np.stack([np.asarray(inp["b_in_re"], np.float64),
                     np.asarray(inp["b_in_im"], np.float64)], axis=1)

    w1, w2 = dw[0, :, 0, :], dw[1, :, 0, :]
    a_taps = np.stack([np.convolve(w2[c], w1[c]) for c in range(MID)])
    a_eff = a_taps * ln_in_w[:, None]
    bias_d = a_taps.sum(1) * ln_in_b + w2.sum(1) * db[0] + db[1]
    e_lo = w2[:, 0] * w1[:, 2]
    e_hi = w2[:, 2] * w1[:, 0]
    e_lo_w = e_lo * ln_in_w
    e_hi_w = e_hi * ln_in_w
    e_lo_b = e_lo * ln_in_b + w2[:, 0] * db[0]
    e_hi_b = e_hi * ln_in_b + w2[:, 2] * db[0]
    edge = np.zeros((128, 12), np.float64)
    edge[:, 0] = -e_lo_w.real
    edge[:, 1] = e_lo_w.imag
    edge[:, 2] = -e_lo_w.imag
    edge[:, 4] = -e_lo_b.real
    edge[:, 5] = -e_lo_b.imag
    edge[:, 6] = -e_hi_w.real
    edge[:, 7] = e_hi_w.imag
    edge[:, 8] = -e_hi_w.imag
    edge[:, 10] = -e_hi_b.real
    edge[:, 11] = -e_hi_b.imag
    db_cc = np.stack([bias_d.real, bias_d.imag], axis=1)

    w_out2 = w_out * ln_out_w[None, :]
    bias_out = w_out @ ln_out_b + b_out
    lA = np.zeros((128, 128), np.float64)
    lA[:, 0:64] = w_out2.real.T
    lA[:, 64:128] = w_out2.imag.T
    lB = np.zeros((128, 128), np.float64)
    lB[:, 0:64] = -w_out2.imag.T
    lB[:, 64:128] = w_out2.real.T
    b_fin = np.concatenate([bias_out.real, bias_out.imag])[:, None]

    return {
        "w_in_r": lin_r.astype(np.float32), "w_in_i": lin_i.astype(np.float32),
        "b_in": b_in.astype(np.float32),
        "a_re": a_eff.real.astype(np.float32), "a_im": a_eff.imag.astype(np.float32),
        "na_im": (-a_eff.imag).astype(np.float32),
        "edge_c": edge.astype(np.float32), "db_c": db_cc.astype(np.float32),
        "w_out_a": lA.astype(bf16), "w_out_b": lB.astype(bf16),
        "b_fin": b_fin.astype(np.float32),
    }


def _make_in_maps(inputs):
    wmap = _prep_weights(inputs)
    x_re = np.asarray(inputs["x_re"], np.float32)
    x_im = np.asarray(inputs["x_im"], np.float32)
    in_maps = []
    for b in range(B):
        x_st = np.concatenate([x_re[b], x_im[b]], axis=0).astype(np.float32)
        m = dict(wmap)
        m["x_in"] = x_st
        in_maps.append(m)
    return in_maps


def kernel(**inputs):
    nc = _get_program()
    in_maps = _make_in_maps(inputs)
    res = run_bass_kernel_spmd(nc, in_maps, core_ids=list(range(B)))
    out = np.empty((2, B, CIN, T), np.float32)
    for b in range(B):
        o = res.results[b]["out"]
        out[0, b] = o[0:64]
        out[1, b] = o[64:128]
    return out

